# revision 23
# baseline (speedup 1.0000x reference)
"""Trainium2 Bass kernel for 16-head MultiHeadAttention (B=2, T=2048, D=1024).

Sharding (8 NeuronCores): core c handles batch b = c//4 and head group
g = c%4 (heads 4g..4g+3).  Each core computes Q/K/V projections for its 4
heads, attention, and a partial output projection against its 256 rows of
W_O.  The host sums the 4 partials per batch and adds b_O (row-parallel TP;
the all-reduce is folded into the unshard step).

Device layout notes:
 - The host pre-transposes x to x^T [D, T] so the contraction dim (features)
   lands on SBUF partitions without any on-device transposes of x.  The 8
   128-row feature chunks are separate SBUF tiles so projection matmuls can
   start as soon as the first chunk's DMA lands.
 - Attention is computed in the S^T = K @ Q^T orientation: the softmax
   denominator is then a partition-axis sum, which the PE produces for free
   via a ones-column appended to V (out = [V|1]^T @ P^T gives O^T rows 0..63
   and the denominator in row 64).
 - V^T is produced directly in [s, dh] orientation by swapping matmul
   operand roles (stationary = x_from^T chunk, moving = Wv), with the bias
   AND the ones-columns injected by one extra K=1 matmul against an
   augmented bias row.  No PE transposes, no ACT copies.
 - The scalar engine runs ONLY the exp stream; everything else lives on
   DVE/Pool so ACT stays at its roofline.
 - Softmax reciprocal uses the fast custom-DVE approx (~5x faster than the
   table-based InstReciprocal) on the [1, 512] denominator rows.
 - Projections and the output projection are interleaved into the attention
   stripes as filler thunks so the PE never idles and stays at high pstate.
   The prologue is minimal (half of K/V, one Q tile) so the exp stream
   starts as early as the input DMA allows.
"""

import os
import sys

from collections import deque

import numpy as np

for _p in ("/opt/trn_rl_repo", "/root/.axon_site/_ro/trn_rl_repo"):
    if os.path.isdir(_p) and _p not in sys.path:
        sys.path.insert(0, _p)

import concourse.bass as bass
import concourse.mybir as mybir
import concourse.tile as tile
from concourse import bacc
from concourse.bass_utils import run_bass_kernel_spmd

F32 = mybir.dt.float32
BF16 = mybir.dt.bfloat16
AF = mybir.ActivationFunctionType

B, TQ, TK = 2, 2048, 2048
D = 1024          # model dim == x_to/x_from feature dim
H, DH = 16, 64
N_CORES = 8
HEADS_PER_CORE = 4   # one batch per core
HP = 2               # head pairs per core (2 heads of 64 stacked -> 128)

TA = 512             # stripe width (queries per stripe)
N_SC = TK // 128     # 16 s-chunks
N_FC = D // 128      # 8 f-chunks

DT = BF16

_CACHED = {}


def build_program():
    nc = bacc.Bacc(
        "TRN2", target_bir_lowering=False, debug=False, num_devices=N_CORES
    )

    xt_to = nc.dram_tensor("xt_to", [D, TQ], DT, kind="ExternalInput")
    xt_from = nc.dram_tensor("xt_from", [D, TK], DT, kind="ExternalInput")
    wq = nc.dram_tensor("wq", [D, 256], DT, kind="ExternalInput")
    wk = nc.dram_tensor("wk", [D, 256], DT, kind="ExternalInput")
    wv = nc.dram_tensor("wv", [D, 260], DT, kind="ExternalInput")
    bq = nc.dram_tensor("bq", [128, 2], F32, kind="ExternalInput")
    bk = nc.dram_tensor("bk", [128, 2], F32, kind="ExternalInput")
    bv = nc.dram_tensor("bv", [1, 260], DT, kind="ExternalInput")
    wot = nc.dram_tensor("wot", [128, 2, 1024], DT, kind="ExternalInput")
    out = nc.dram_tensor("out", [TQ, D], DT, kind="ExternalOutput")

    with tile.TileContext(nc) as tc:
        with (
            tc.tile_pool(name="wpool", bufs=1) as wpool,
            tc.tile_pool(name="actpool", bufs=1) as actpool,
            tc.tile_pool(name="ptpool", bufs=3) as ptpool,
            tc.tile_pool(name="misc", bufs=2) as misc,
            tc.tile_pool(name="psmm", bufs=2, space="PSUM") as psmm,
            tc.tile_pool(name="psacc", bufs=2, space="PSUM") as psacc,
            tc.tile_pool(name="psaux", bufs=2, space="PSUM") as psaux,
        ):
            # ---- weights / constants -------------------------------------
            wq_sb = wpool.tile([128, N_FC, 256], DT)
            wk_sb = wpool.tile([128, N_FC, 256], DT)
            wv_sb = wpool.tile([128, N_FC, 260], DT)
            bq_sb = wpool.tile([128, 2], F32)
            bk_sb = wpool.tile([128, 2], F32)
            bv_sb = wpool.tile([1, 260], DT)
            wot_sb = wpool.tile([128, 2, 1024], DT)
            ones_sb = wpool.tile([1, 128], DT)
            nc.vector.memset(ones_sb[:], 1.0)

            # x^T chunk-pair tiles -> DMA->fill-thunk deps at matching
            # granularity (each fill thunk consumes one fc pair)
            xfr_sb = [
                actpool.tile([128, 2, TK], DT, name=f"xfr{fp}")
                for fp in range(N_FC // 2)
            ]
            xto_sb = [
                actpool.tile([128, 2, TQ], DT, name=f"xto{fp}")
                for fp in range(N_FC // 2)
            ]
            xt_to_r = xt_to.rearrange("(c p) t -> p c t", p=128)
            xt_from_r = xt_from.rearrange("(c p) t -> p c t", p=128)

            # Input DMAs split across BOTH hardware DGE queues: the x_from
            # (K/V) stream issues from Sync, the x_to (Q) stream from the
            # Scalar engine (idle until the first exp anyway), so the two
            # 4MB streams transfer in parallel.  Biases trail the x data.
            nc.sync.dma_start(wk_sb[:], wk.rearrange("(c p) d -> p c d", p=128))
            nc.sync.dma_start(wv_sb[:], wv.rearrange("(c p) d -> p c d", p=128))
            nc.scalar.dma_start(
                wq_sb[:], wq.rearrange("(c p) d -> p c d", p=128)
            )
            for fp in range(N_FC // 2):
                nc.sync.dma_start(
                    xfr_sb[fp][:], xt_from_r[:, 2 * fp:2 * fp + 2, :]
                )
                nc.scalar.dma_start(
                    xto_sb[fp][:], xt_to_r[:, 2 * fp:2 * fp + 2, :]
                )
            nc.sync.dma_start(bk_sb[:], bk[:])
            nc.sync.dma_start(bv_sb[:], bv[:])
            nc.scalar.dma_start(bq_sb[:], bq[:])
            nc.scalar.dma_start(wot_sb[:], wot[:])

            # ---- persistent activations ----------------------------------
            qt_sb = [
                actpool.tile([128, TQ], DT, name=f"qt{hp}") for hp in range(HP)
            ]
            kt_sb = [
                actpool.tile([128, TK], DT, name=f"kt{hp}") for hp in range(HP)
            ]
            # V^T with ones columns: head h at cols 65h..65h+63, ones at
            # 65h+64 (4 heads -> 260 cols), per 128-wide s-chunk
            vn_sb = actpool.tile([128, N_SC, 260], DT, name="vn_sb")
            ot_sb = [
                actpool.tile([128, TQ], DT, name=f"ot{hp}") for hp in range(HP)
            ]

            # ---- thunk emitters ------------------------------------------
            def qk_thunks(w_sb, b_sb, x_sb, dst, hp, tt):
                """Q/K projection for one [128, 1024] tile: two psum halves,
                each 8 accumulating matmuls + a bias-add copyback."""
                thunks = []
                dsl = bass.ts(hp, 128)
                for half in range(2):
                    ps = psaux.tile([128, 512], F32, name="ps_x")
                    t0 = tt * 1024 + half * 512
                    for fcp in range(N_FC // 2):
                        def fill(fcp=fcp, ps=ps, t0=t0):
                            for k in range(2):
                                fc = 2 * fcp + k
                                nc.tensor.matmul(
                                    ps[:],
                                    w_sb[:, fc, dsl],
                                    x_sb[fcp][:, k, t0:t0 + 512],
                                    start=(fc == 0),
                                    stop=(fc == N_FC - 1),
                                )
                        thunks.append(fill)

                    def copyback(ps=ps, t0=t0):
                        nc.vector.tensor_scalar_add(
                            dst[hp][:, t0:t0 + 512], ps[:], b_sb[:, hp:hp + 1]
                        )
                    thunks.append(copyback)
                return thunks

            def v_thunks(sc):
                """V^T for one s-chunk, computed directly in [s, dh]
                orientation: stationary = x_from^T chunk, moving = Wv.
                Bias + ones columns injected via a K=1 matmul."""
                thunks = []
                ps = psaux.tile([128, 512], F32, name="ps_x")
                ssl = bass.ts(sc, 128)
                for fcp in range(N_FC // 2):
                    def fill(fcp=fcp, ps=ps):
                        for k in range(2):
                            fc = 2 * fcp + k
                            nc.tensor.matmul(
                                ps[:, 0:260],
                                xfr_sb[fcp][:, k, ssl],
                                wv_sb[:, fc, :],
                                start=(fc == 0),
                                stop=False,
                            )
                    thunks.append(fill)

                def bias(ps=ps):
                    nc.tensor.matmul(
                        ps[:, 0:260],
                        ones_sb[:],
                        bv_sb[:],
                        start=False,
                        stop=True,
                    )
                thunks.append(bias)

                def copyback(ps=ps):
                    nc.vector.tensor_copy(vn_sb[:, sc, :], ps[:, 0:260])
                thunks.append(copyback)
                return thunks

            out_r = out.rearrange("(a p) d -> p a d", p=128)

            def outproj_thunks(tta):
                """Output projection for one stripe of queries: 4 t-chunks
                of 128, each = 2 psum halves (contraction over both head
                pairs) + copyback into a stripe-wide staging tile, then one
                batched DMA for all 512 rows."""
                thunks = []
                o_t = misc.tile([128, TA // 128, 1024], DT, name="o_t")
                for j in range(TA // 128):
                    tc_ = tta * (TA // 128) + j
                    tsl = bass.ts(tc_, 128)
                    for half in range(2):
                        ps = psaux.tile([128, 512], F32, name="ps_x")
                        hsl = bass.ts(half, 512)

                        def mmf(ps=ps, tsl=tsl, hsl=hsl):
                            for hp in range(HP):
                                nc.tensor.matmul(
                                    ps[:],
                                    ot_sb[hp][:, tsl],
                                    wot_sb[:, hp, hsl],
                                    start=(hp == 0),
                                    stop=(hp == HP - 1),
                                )
                        thunks.append(mmf)

                        def cb(ps=ps, j=j, hsl=hsl):
                            nc.vector.tensor_copy(o_t[:, j, hsl], ps[:])
                        thunks.append(cb)

                def store():
                    nc.sync.dma_start(
                        out_r[:, 4 * tta:4 * tta + 4, :], o_t[:]
                    )
                thunks.append(store)
                return thunks

            def emit_stripe(tta, hp, pop_filler, per_iter):
                """One attention stripe: both heads of the pair, 512
                queries, all 2048 keys.  Pops filler thunks per s-chunk so
                independent PE work interleaves with the ACT exp stream.
                ps_o is split per head on a bufs=2 ring so the next
                stripe's PV only waits on the matching head's drain."""
                ps_o = [
                    psacc.tile([65, TA], F32, name="ps_o") for _ in range(2)
                ]
                for sc in range(N_SC):
                    ps_s = psmm.tile([128, 1024], F32, name="ps_s")
                    for h in range(2):
                        hb = 64 * h
                        nc.tensor.matmul(
                            ps_s[:, bass.ts(h, TA)],
                            kt_sb[hp][hb:hb + 64, bass.ts(sc, 128)],
                            qt_sb[hp][hb:hb + 64, bass.ts(tta, TA)],
                            start=True,
                            stop=True,
                        )
                    pt = ptpool.tile([128, 1024], DT, name="pt")
                    nc.scalar.activation(pt[:], ps_s[:], AF.Exp)
                    for h in range(2):
                        vb = 65 * (2 * hp + h)
                        nc.tensor.matmul(
                            ps_o[h][:],
                            vn_sb[:, sc, vb:vb + 65],
                            pt[:, bass.ts(h, TA)],
                            start=(sc == 0),
                            stop=(sc == N_SC - 1),
                        )
                    pop_filler(per_iter)

                # denominators (psum row 64) -> fast reciprocal -> broadcast
                # across partitions (Pool) -> normalize ps_o into ot (DVE)
                # (reciprocal_approx_fast must NOT read PSUM directly)
                recs = []
                for h in range(2):
                    rec = misc.tile([1, TA], F32, name="rec_t")
                    nc.vector.tensor_copy(rec[:], ps_o[h][64:65, :])
                    nc.vector.reciprocal_approx_fast(rec[:], rec[:])
                    recs.append(rec)
                for h in range(2):
                    r_sb = misc.tile([128, TA], F32, name="r_sb")
                    nc.gpsimd.partition_broadcast(r_sb[:], recs[h][:])
                    hb = 64 * h
                    nc.vector.tensor_mul(
                        ot_sb[hp][hb:hb + 64, bass.ts(tta, TA)],
                        ps_o[h][0:64, :],
                        r_sb[0:64, :],
                    )

            # ---- emission schedule ---------------------------------------
            # Minimal prologue so the exp stream starts as early as the
            # input DMA allows: K(hp0,tt0) covers keys for s-chunks 0..7,
            # V(0..7), Q(hp0,tt0).  Everything else (K tt1, V 8..15, the
            # other head pair, remaining Q tiles, output projections)
            # streams in as filler thunks inside the stripes.  A stripe's
            # own s-chunk loop only consumes V(sc)/K(tt1) at iteration
            # sc >= 8, by which point the first stripe's high filler rate
            # has emitted them.  Anything a stripe needs at its FIRST
            # iteration is force-drained before the stripe starts.
            for f in qk_thunks(wk_sb, bk_sb, xfr_sb, kt_sb, 0, 0):
                f()
            for sc in range(8):
                for f in v_thunks(sc):
                    f()
            for f in qk_thunks(wq_sb, bq_sb, xto_sb, qt_sb, 0, 0):
                f()

            fillers = deque()
            fillers.extend(qk_thunks(wk_sb, bk_sb, xfr_sb, kt_sb, 0, 1))
            for sc in range(8, N_SC):
                fillers.extend(v_thunks(sc))
            fillers.extend(qk_thunks(wk_sb, bk_sb, xfr_sb, kt_sb, 1, 0))
            fillers.extend(qk_thunks(wk_sb, bk_sb, xfr_sb, kt_sb, 1, 1))
            fillers.extend(qk_thunks(wq_sb, bq_sb, xto_sb, qt_sb, 1, 0))
            n_before_01 = len(fillers)
            fillers.extend(qk_thunks(wq_sb, bq_sb, xto_sb, qt_sb, 0, 1))
            n_before_10 = len(fillers)
            fillers.extend(qk_thunks(wq_sb, bq_sb, xto_sb, qt_sb, 1, 1))
            n_before_11 = len(fillers)

            popped = [0]

            def pop_filler(n):
                for _ in range(n):
                    if fillers:
                        fillers.popleft()()
                        popped[0] += 1

            def drain_to(target):
                while fillers and popped[0] < target:
                    fillers.popleft()()
                    popped[0] += 1

            per_iter_schedule = {
                (0, 0): 7, (0, 1): 3, (1, 0): 3, (1, 1): 2,
                (2, 0): 2, (2, 1): 2, (3, 0): 2, (3, 1): 2,
            }
            guards = {(0, 1): n_before_01, (1, 0): n_before_10,
                      (1, 1): n_before_11}

            for tta in range(TQ // TA):
                for hp in range(HP):
                    drain_to(guards.get((tta, hp), 0))
                    emit_stripe(tta, hp, pop_filler,
                                per_iter_schedule[(tta, hp)])
                fillers.extend(outproj_thunks(tta))

            while fillers:
                fillers.popleft()()

    nc.compile()
    return nc


def _prep_in_maps(x_to, x_from, Wq, bq, Wk, bk, Wv, bv, Wo):
    scale = 1.0 / np.sqrt(np.float32(DH))
    # [H, D, DH] -> [D, H*DH] with column h*DH+d
    wq_f = np.ascontiguousarray(Wq.transpose(1, 0, 2).reshape(D, H * DH)) * scale
    wk_f = np.ascontiguousarray(Wk.transpose(1, 0, 2).reshape(D, H * DH))
    bq_f = bq.reshape(H * DH) * scale
    bk_f = bk.reshape(H * DH)

    xt_to = np.ascontiguousarray(x_to.transpose(0, 2, 1))    # [B, D, TQ]
    xt_from = np.ascontiguousarray(x_from.transpose(0, 2, 1))

    def f32(a):
        return np.ascontiguousarray(a, dtype=np.float32)

    import ml_dtypes

    def fdt(a):
        return np.ascontiguousarray(a, dtype=ml_dtypes.bfloat16)

    in_maps = []
    for c in range(N_CORES):
        b, g = divmod(c, HEADS_PER_CORE)
        cs = slice(g * 256, (g + 1) * 256)
        # Wv augmented: head h (of the core's 4) at cols 65h..65h+63,
        # zero col at 65h+64; bias row gets bv there plus 1.0 ones
        wv_aug = np.zeros((D, 260), dtype=np.float32)
        bv_aug = np.zeros((260,), dtype=np.float32)
        for h in range(4):
            head = 4 * g + h
            wv_aug[:, 65 * h:65 * h + 64] = Wv[head]
            bv_aug[65 * h:65 * h + 64] = bv[head]
            bv_aug[65 * h + 64] = 1.0
        in_maps.append(
            {
                "xt_to": fdt(xt_to[b]),
                "xt_from": fdt(xt_from[b]),
                "wq": fdt(wq_f[:, cs]),
                "wk": fdt(wk_f[:, cs]),
                "wv": fdt(wv_aug),
                # [256] -> [2 pairs, 128] -> [128, 2]
                "bq": f32(bq_f[cs].reshape(2, 128).T),
                "bk": f32(bk_f[cs].reshape(2, 128).T),
                "bv": fdt(bv_aug.reshape(1, 260)),
                # Wo[:, cs].T = [256, 1024] -> [2, 128, 1024] -> [128, 2, 1024]
                "wot": fdt(
                    np.ascontiguousarray(Wo[:, cs].T)
                    .reshape(2, 128, 1024)
                    .transpose(1, 0, 2)
                ),
            }
        )
    return in_maps


LAST_EXEC_TIME_NS = None
LAST_TRACE = None


def kernel(x_to, x_from, Wq, bq, Wk, bk, Wv, bv, Wo, bo):
    global LAST_EXEC_TIME_NS, LAST_TRACE
    if "nc" not in _CACHED:
        _CACHED["nc"] = build_program()
    nc = _CACHED["nc"]

    in_maps = _prep_in_maps(
        np.asarray(x_to), np.asarray(x_from), np.asarray(Wq), np.asarray(bq),
        np.asarray(Wk), np.asarray(bk), np.asarray(Wv), np.asarray(bv),
        np.asarray(Wo),
    )
    res = run_bass_kernel_spmd(nc, in_maps, list(range(N_CORES)))
    LAST_EXEC_TIME_NS = res.exec_time_ns
    LAST_TRACE = res.instructions_and_trace

    out = np.zeros((B, TQ, D), dtype=np.float32)
    for c in range(N_CORES):
        out[c // HEADS_PER_CORE] += np.asarray(
            res.results[c]["out"], dtype=np.float32
        )
    out += np.asarray(bo, dtype=np.float32)
    return out


# revision 26
# speedup vs baseline: 1.0037x; 1.0037x over previous
"""Trainium2 Bass kernel for 16-head MultiHeadAttention (B=2, T=2048, D=1024).

Sharding (8 NeuronCores): core c handles batch b = c//4 and head group
g = c%4 (heads 4g..4g+3).  Each core computes Q/K/V projections for its 4
heads, attention, and a partial output projection against its 256 rows of
W_O.  The host sums the 4 partials per batch and adds b_O (row-parallel TP;
the all-reduce is folded into the unshard step).

Device layout notes:
 - The host pre-transposes x to x^T [D, T] so the contraction dim (features)
   lands on SBUF partitions without any on-device transposes of x.  The 8
   128-row feature chunks are separate SBUF tiles so projection matmuls can
   start as soon as the first chunk's DMA lands.
 - Attention is computed in the S^T = K @ Q^T orientation: the softmax
   denominator is then a partition-axis sum, which the PE produces for free
   via a ones-column appended to V (out = [V|1]^T @ P^T gives O^T rows 0..63
   and the denominator in row 64).
 - V^T is produced directly in [s, dh] orientation by swapping matmul
   operand roles (stationary = x_from^T chunk, moving = Wv), with the bias
   AND the ones-columns injected by one extra K=1 matmul against an
   augmented bias row.  No PE transposes, no ACT copies.
 - The scalar engine runs ONLY the exp stream; everything else lives on
   DVE/Pool so ACT stays at its roofline.
 - Softmax reciprocal uses the fast custom-DVE approx (~5x faster than the
   table-based InstReciprocal) on the [1, 512] denominator rows.
 - Projections and the output projection are interleaved into the attention
   stripes as filler thunks so the PE never idles and stays at high pstate.
   The prologue is minimal (half of K/V, one Q tile) so the exp stream
   starts as early as the input DMA allows.
"""

import os
import sys

from collections import deque

import numpy as np

for _p in ("/opt/trn_rl_repo", "/root/.axon_site/_ro/trn_rl_repo"):
    if os.path.isdir(_p) and _p not in sys.path:
        sys.path.insert(0, _p)

import concourse.bass as bass
import concourse.mybir as mybir
import concourse.tile as tile
from concourse import bacc
from concourse.bass_utils import run_bass_kernel_spmd

F32 = mybir.dt.float32
BF16 = mybir.dt.bfloat16
AF = mybir.ActivationFunctionType

B, TQ, TK = 2, 2048, 2048
D = 1024          # model dim == x_to/x_from feature dim
H, DH = 16, 64
N_CORES = 8
HEADS_PER_CORE = 4   # one batch per core
HP = 2               # head pairs per core (2 heads of 64 stacked -> 128)

TA = 512             # stripe width (queries per stripe)
N_SC = TK // 128     # 16 s-chunks
N_FC = D // 128      # 8 f-chunks

DT = BF16

_CACHED = {}


def build_program():
    nc = bacc.Bacc(
        "TRN2", target_bir_lowering=False, debug=False, num_devices=N_CORES
    )

    xt_to = nc.dram_tensor("xt_to", [D, TQ], DT, kind="ExternalInput")
    xt_from = nc.dram_tensor("xt_from", [D, TK], DT, kind="ExternalInput")
    # weights already in partition-major [128, fc, d] layout host-side so
    # the DMA moves 4KB-contiguous per-partition lines (fast packets)
    wq = nc.dram_tensor("wq", [128, N_FC, 256], DT, kind="ExternalInput")
    wk = nc.dram_tensor("wk", [128, N_FC, 256], DT, kind="ExternalInput")
    wv = nc.dram_tensor("wv", [128, N_FC, 260], DT, kind="ExternalInput")
    bq = nc.dram_tensor("bq", [128, 2], F32, kind="ExternalInput")
    bk = nc.dram_tensor("bk", [128, 2], F32, kind="ExternalInput")
    bv = nc.dram_tensor("bv", [1, 260], DT, kind="ExternalInput")
    wot = nc.dram_tensor("wot", [128, 2, 1024], DT, kind="ExternalInput")
    out = nc.dram_tensor("out", [TQ, D], DT, kind="ExternalOutput")

    with tile.TileContext(nc) as tc:
        with (
            tc.tile_pool(name="wpool", bufs=1) as wpool,
            tc.tile_pool(name="actpool", bufs=1) as actpool,
            tc.tile_pool(name="ptpool", bufs=3) as ptpool,
            tc.tile_pool(name="misc", bufs=2) as misc,
            tc.tile_pool(name="psmm", bufs=2, space="PSUM") as psmm,
            tc.tile_pool(name="psacc", bufs=2, space="PSUM") as psacc,
            tc.tile_pool(name="psaux", bufs=2, space="PSUM") as psaux,
        ):
            # ---- weights / constants -------------------------------------
            wq_sb = wpool.tile([128, N_FC, 256], DT)
            wk_sb = wpool.tile([128, N_FC, 256], DT)
            wv_sb = wpool.tile([128, N_FC, 260], DT)
            bq_sb = wpool.tile([128, 2], F32)
            bk_sb = wpool.tile([128, 2], F32)
            bv_sb = wpool.tile([1, 260], DT)
            wot_sb = wpool.tile([128, 2, 1024], DT)
            ones_sb = wpool.tile([1, 128], DT)
            nc.vector.memset(ones_sb[:], 1.0)

            # x^T chunk-pair tiles -> DMA->fill-thunk deps at matching
            # granularity (each fill thunk consumes one fc pair)
            xfr_sb = [
                actpool.tile([128, 2, TK], DT, name=f"xfr{fp}")
                for fp in range(N_FC // 2)
            ]
            xto_sb = [
                actpool.tile([128, 2, TQ], DT, name=f"xto{fp}")
                for fp in range(N_FC // 2)
            ]
            xt_to_r = xt_to.rearrange("(c p) t -> p c t", p=128)
            xt_from_r = xt_from.rearrange("(c p) t -> p c t", p=128)

            # Input DMAs split across BOTH hardware DGE queues: the x_from
            # (K/V) stream issues from Sync, the x_to (Q) stream from the
            # Scalar engine (idle until the first exp anyway), so the two
            # 4MB streams transfer in parallel.  Biases trail the x data.
            nc.sync.dma_start(wk_sb[:], wk[:])
            nc.sync.dma_start(wv_sb[:], wv[:])
            nc.scalar.dma_start(wq_sb[:], wq[:])
            for fp in range(N_FC // 2):
                nc.sync.dma_start(
                    xfr_sb[fp][:], xt_from_r[:, 2 * fp:2 * fp + 2, :]
                )
                nc.scalar.dma_start(
                    xto_sb[fp][:], xt_to_r[:, 2 * fp:2 * fp + 2, :]
                )
            nc.sync.dma_start(bk_sb[:], bk[:])
            nc.sync.dma_start(bv_sb[:], bv[:])
            nc.scalar.dma_start(bq_sb[:], bq[:])
            nc.scalar.dma_start(wot_sb[:], wot[:])

            # ---- persistent activations ----------------------------------
            qt_sb = [
                actpool.tile([128, TQ], DT, name=f"qt{hp}") for hp in range(HP)
            ]
            kt_sb = [
                actpool.tile([128, TK], DT, name=f"kt{hp}") for hp in range(HP)
            ]
            # V^T with ones columns: head h at cols 65h..65h+63, ones at
            # 65h+64 (4 heads -> 260 cols), per 128-wide s-chunk
            vn_sb = actpool.tile([128, N_SC, 260], DT, name="vn_sb")
            ot_sb = [
                actpool.tile([128, TQ], DT, name=f"ot{hp}") for hp in range(HP)
            ]

            # ---- thunk emitters ------------------------------------------
            def qk_thunks(w_sb, b_sb, x_sb, dst, hp, tt):
                """Q/K projection for one [128, 1024] tile: two psum halves,
                each 8 accumulating matmuls + a bias-add copyback."""
                thunks = []
                dsl = bass.ts(hp, 128)
                for half in range(2):
                    ps = psaux.tile([128, 512], F32, name="ps_x")
                    t0 = tt * 1024 + half * 512
                    for fcp in range(N_FC // 2):
                        def fill(fcp=fcp, ps=ps, t0=t0):
                            for k in range(2):
                                fc = 2 * fcp + k
                                nc.tensor.matmul(
                                    ps[:],
                                    w_sb[:, fc, dsl],
                                    x_sb[fcp][:, k, t0:t0 + 512],
                                    start=(fc == 0),
                                    stop=(fc == N_FC - 1),
                                )
                        thunks.append(fill)

                    def copyback(ps=ps, t0=t0):
                        nc.vector.tensor_scalar_add(
                            dst[hp][:, t0:t0 + 512], ps[:], b_sb[:, hp:hp + 1]
                        )
                    thunks.append(copyback)
                return thunks

            def v_thunks(sc):
                """V^T for one s-chunk, computed directly in [s, dh]
                orientation: stationary = x_from^T chunk, moving = Wv.
                Bias + ones columns injected via a K=1 matmul."""
                thunks = []
                ps = psaux.tile([128, 512], F32, name="ps_x")
                ssl = bass.ts(sc, 128)
                for fcp in range(N_FC // 2):
                    def fill(fcp=fcp, ps=ps):
                        for k in range(2):
                            fc = 2 * fcp + k
                            nc.tensor.matmul(
                                ps[:, 0:260],
                                xfr_sb[fcp][:, k, ssl],
                                wv_sb[:, fc, :],
                                start=(fc == 0),
                                stop=False,
                            )
                    thunks.append(fill)

                def bias(ps=ps):
                    nc.tensor.matmul(
                        ps[:, 0:260],
                        ones_sb[:],
                        bv_sb[:],
                        start=False,
                        stop=True,
                    )
                thunks.append(bias)

                def copyback(ps=ps):
                    nc.vector.tensor_copy(vn_sb[:, sc, :], ps[:, 0:260])
                thunks.append(copyback)
                return thunks

            out_r = out.rearrange("(a p) d -> p a d", p=128)

            def outproj_thunks(tta):
                """Output projection for one stripe of queries: 4 t-chunks
                of 128, each = 2 psum halves (contraction over both head
                pairs) + copyback into a stripe-wide staging tile, then one
                batched DMA for all 512 rows."""
                thunks = []
                o_t = misc.tile([128, TA // 128, 1024], DT, name="o_t")
                for j in range(TA // 128):
                    tc_ = tta * (TA // 128) + j
                    tsl = bass.ts(tc_, 128)
                    for half in range(2):
                        ps = psaux.tile([128, 512], F32, name="ps_x")
                        hsl = bass.ts(half, 512)

                        def mmf(ps=ps, tsl=tsl, hsl=hsl):
                            for hp in range(HP):
                                nc.tensor.matmul(
                                    ps[:],
                                    ot_sb[hp][:, tsl],
                                    wot_sb[:, hp, hsl],
                                    start=(hp == 0),
                                    stop=(hp == HP - 1),
                                )
                        thunks.append(mmf)

                        def cb(ps=ps, j=j, hsl=hsl):
                            nc.vector.tensor_copy(o_t[:, j, hsl], ps[:])
                        thunks.append(cb)

                def store():
                    nc.sync.dma_start(
                        out_r[:, 4 * tta:4 * tta + 4, :], o_t[:]
                    )
                thunks.append(store)
                return thunks

            def emit_stripe(tta, hp, pop_filler, per_iter):
                """One attention stripe: both heads of the pair, 512
                queries, all 2048 keys.  Pops filler thunks per s-chunk so
                independent PE work interleaves with the ACT exp stream.
                ps_o is split per head on a bufs=2 ring so the next
                stripe's PV only waits on the matching head's drain."""
                ps_o = [
                    psacc.tile([65, TA], F32, name="ps_o") for _ in range(2)
                ]
                for sc in range(N_SC):
                    ps_s = psmm.tile([128, 1024], F32, name="ps_s")
                    for h in range(2):
                        hb = 64 * h
                        nc.tensor.matmul(
                            ps_s[:, bass.ts(h, TA)],
                            kt_sb[hp][hb:hb + 64, bass.ts(sc, 128)],
                            qt_sb[hp][hb:hb + 64, bass.ts(tta, TA)],
                            start=True,
                            stop=True,
                        )
                    pt = ptpool.tile([128, 1024], DT, name="pt")
                    nc.scalar.activation(pt[:], ps_s[:], AF.Exp)
                    for h in range(2):
                        vb = 65 * (2 * hp + h)
                        nc.tensor.matmul(
                            ps_o[h][:],
                            vn_sb[:, sc, vb:vb + 65],
                            pt[:, bass.ts(h, TA)],
                            start=(sc == 0),
                            stop=(sc == N_SC - 1),
                        )
                    pop_filler(per_iter)

                # denominators (psum row 64) -> fast reciprocal -> broadcast
                # across partitions (Pool) -> normalize ps_o into ot (DVE)
                # (reciprocal_approx_fast must NOT read PSUM directly)
                recs = []
                for h in range(2):
                    rec = misc.tile([1, TA], F32, name="rec_t")
                    nc.vector.tensor_copy(rec[:], ps_o[h][64:65, :])
                    nc.vector.reciprocal_approx_fast(rec[:], rec[:])
                    recs.append(rec)
                for h in range(2):
                    r_sb = misc.tile([128, TA], F32, name="r_sb")
                    nc.gpsimd.partition_broadcast(r_sb[:], recs[h][:])
                    hb = 64 * h
                    nc.vector.tensor_mul(
                        ot_sb[hp][hb:hb + 64, bass.ts(tta, TA)],
                        ps_o[h][0:64, :],
                        r_sb[0:64, :],
                    )

            # ---- emission schedule ---------------------------------------
            # Minimal prologue so the exp stream starts as early as the
            # input DMA allows: K(hp0,tt0) covers keys for s-chunks 0..7,
            # V(0..7), Q(hp0,tt0).  Everything else (K tt1, V 8..15, the
            # other head pair, remaining Q tiles, output projections)
            # streams in as filler thunks inside the stripes.  A stripe's
            # own s-chunk loop only consumes V(sc)/K(tt1) at iteration
            # sc >= 8, by which point the first stripe's high filler rate
            # has emitted them.  Anything a stripe needs at its FIRST
            # iteration is force-drained before the stripe starts.
            for f in qk_thunks(wk_sb, bk_sb, xfr_sb, kt_sb, 0, 0):
                f()
            for sc in range(8):
                for f in v_thunks(sc):
                    f()
            for f in qk_thunks(wq_sb, bq_sb, xto_sb, qt_sb, 0, 0):
                f()

            fillers = deque()
            fillers.extend(qk_thunks(wk_sb, bk_sb, xfr_sb, kt_sb, 0, 1))
            for sc in range(8, N_SC):
                fillers.extend(v_thunks(sc))
            fillers.extend(qk_thunks(wk_sb, bk_sb, xfr_sb, kt_sb, 1, 0))
            fillers.extend(qk_thunks(wk_sb, bk_sb, xfr_sb, kt_sb, 1, 1))
            fillers.extend(qk_thunks(wq_sb, bq_sb, xto_sb, qt_sb, 1, 0))
            n_before_01 = len(fillers)
            fillers.extend(qk_thunks(wq_sb, bq_sb, xto_sb, qt_sb, 0, 1))
            n_before_10 = len(fillers)
            fillers.extend(qk_thunks(wq_sb, bq_sb, xto_sb, qt_sb, 1, 1))
            n_before_11 = len(fillers)

            popped = [0]

            def pop_filler(n):
                for _ in range(n):
                    if fillers:
                        fillers.popleft()()
                        popped[0] += 1

            def drain_to(target):
                while fillers and popped[0] < target:
                    fillers.popleft()()
                    popped[0] += 1

            per_iter_schedule = {
                (0, 0): 7, (0, 1): 3, (1, 0): 3, (1, 1): 2,
                (2, 0): 2, (2, 1): 2, (3, 0): 2, (3, 1): 2,
            }
            guards = {(0, 1): n_before_01, (1, 0): n_before_10,
                      (1, 1): n_before_11}

            for tta in range(TQ // TA):
                for hp in range(HP):
                    drain_to(guards.get((tta, hp), 0))
                    emit_stripe(tta, hp, pop_filler,
                                per_iter_schedule[(tta, hp)])
                fillers.extend(outproj_thunks(tta))

            while fillers:
                fillers.popleft()()

    nc.compile()
    return nc


def _prep_in_maps(x_to, x_from, Wq, bq, Wk, bk, Wv, bv, Wo):
    scale = 1.0 / np.sqrt(np.float32(DH))
    # [H, D, DH] -> [D, H*DH] with column h*DH+d
    wq_f = np.ascontiguousarray(Wq.transpose(1, 0, 2).reshape(D, H * DH)) * scale
    wk_f = np.ascontiguousarray(Wk.transpose(1, 0, 2).reshape(D, H * DH))
    bq_f = bq.reshape(H * DH) * scale
    bk_f = bk.reshape(H * DH)

    xt_to = np.ascontiguousarray(x_to.transpose(0, 2, 1))    # [B, D, TQ]
    xt_from = np.ascontiguousarray(x_from.transpose(0, 2, 1))

    def f32(a):
        return np.ascontiguousarray(a, dtype=np.float32)

    import ml_dtypes

    def fdt(a):
        return np.ascontiguousarray(a, dtype=ml_dtypes.bfloat16)

    in_maps = []
    for c in range(N_CORES):
        b, g = divmod(c, HEADS_PER_CORE)
        cs = slice(g * 256, (g + 1) * 256)
        # Wv augmented: head h (of the core's 4) at cols 65h..65h+63,
        # zero col at 65h+64; bias row gets bv there plus 1.0 ones
        wv_aug = np.zeros((D, 260), dtype=np.float32)
        bv_aug = np.zeros((260,), dtype=np.float32)
        for h in range(4):
            head = 4 * g + h
            wv_aug[:, 65 * h:65 * h + 64] = Wv[head]
            bv_aug[65 * h:65 * h + 64] = bv[head]
            bv_aug[65 * h + 64] = 1.0
        def pmajor(w):
            # [D, d] -> [128, N_FC, d] partition-major
            return np.ascontiguousarray(
                w.reshape(N_FC, 128, w.shape[1]).transpose(1, 0, 2)
            )

        in_maps.append(
            {
                "xt_to": fdt(xt_to[b]),
                "xt_from": fdt(xt_from[b]),
                "wq": fdt(pmajor(wq_f[:, cs])),
                "wk": fdt(pmajor(wk_f[:, cs])),
                "wv": fdt(pmajor(wv_aug)),
                # [256] -> [2 pairs, 128] -> [128, 2]
                "bq": f32(bq_f[cs].reshape(2, 128).T),
                "bk": f32(bk_f[cs].reshape(2, 128).T),
                "bv": fdt(bv_aug.reshape(1, 260)),
                # Wo[:, cs].T = [256, 1024] -> [2, 128, 1024] -> [128, 2, 1024]
                "wot": fdt(
                    np.ascontiguousarray(Wo[:, cs].T)
                    .reshape(2, 128, 1024)
                    .transpose(1, 0, 2)
                ),
            }
        )
    return in_maps


LAST_EXEC_TIME_NS = None
LAST_TRACE = None


def kernel(x_to, x_from, Wq, bq, Wk, bk, Wv, bv, Wo, bo):
    global LAST_EXEC_TIME_NS, LAST_TRACE
    if "nc" not in _CACHED:
        _CACHED["nc"] = build_program()
    nc = _CACHED["nc"]

    in_maps = _prep_in_maps(
        np.asarray(x_to), np.asarray(x_from), np.asarray(Wq), np.asarray(bq),
        np.asarray(Wk), np.asarray(bk), np.asarray(Wv), np.asarray(bv),
        np.asarray(Wo),
    )
    res = run_bass_kernel_spmd(nc, in_maps, list(range(N_CORES)))
    LAST_EXEC_TIME_NS = res.exec_time_ns
    LAST_TRACE = res.instructions_and_trace

    out = np.zeros((B, TQ, D), dtype=np.float32)
    for c in range(N_CORES):
        out[c // HEADS_PER_CORE] += np.asarray(
            res.results[c]["out"], dtype=np.float32
        )
    out += np.asarray(bo, dtype=np.float32)
    return out


# revision 29
# speedup vs baseline: 1.0073x; 1.0037x over previous
"""Trainium2 Bass kernel for 16-head MultiHeadAttention (B=2, T=2048, D=1024).

Sharding (8 NeuronCores): core c handles batch b = c//4 and head group
g = c%4 (heads 4g..4g+3).  Each core computes Q/K/V projections for its 4
heads, attention, and a partial output projection against its 256 rows of
W_O.  The host sums the 4 partials per batch and adds b_O (row-parallel TP;
the all-reduce is folded into the unshard step).

Device layout notes:
 - The host pre-transposes x to x^T [D, T] so the contraction dim (features)
   lands on SBUF partitions without any on-device transposes of x.  The 8
   128-row feature chunks are separate SBUF tiles so projection matmuls can
   start as soon as the first chunk's DMA lands.
 - Attention is computed in the S^T = K @ Q^T orientation: the softmax
   denominator is then a partition-axis sum, which the PE produces for free
   via a ones-column appended to V (out = [V|1]^T @ P^T gives O^T rows 0..63
   and the denominator in row 64).
 - V^T is produced directly in [s, dh] orientation by swapping matmul
   operand roles (stationary = x_from^T chunk, moving = Wv), with the bias
   AND the ones-columns injected by one extra K=1 matmul against an
   augmented bias row.  No PE transposes, no ACT copies.
 - The scalar engine runs ONLY the exp stream; everything else lives on
   DVE/Pool so ACT stays at its roofline.
 - Softmax reciprocal uses the fast custom-DVE approx (~5x faster than the
   table-based InstReciprocal) on the [1, 512] denominator rows.
 - Projections and the output projection are interleaved into the attention
   stripes as filler thunks so the PE never idles and stays at high pstate.
   The prologue is minimal (half of K/V, one Q tile) so the exp stream
   starts as early as the input DMA allows.
"""

import os
import sys

from collections import deque

import numpy as np

for _p in ("/opt/trn_rl_repo", "/root/.axon_site/_ro/trn_rl_repo"):
    if os.path.isdir(_p) and _p not in sys.path:
        sys.path.insert(0, _p)

import concourse.bass as bass
import concourse.mybir as mybir
import concourse.tile as tile
from concourse import bacc
from concourse.bass_utils import run_bass_kernel_spmd

F32 = mybir.dt.float32
BF16 = mybir.dt.bfloat16
AF = mybir.ActivationFunctionType

B, TQ, TK = 2, 2048, 2048
D = 1024          # model dim == x_to/x_from feature dim
H, DH = 16, 64
N_CORES = 8
HEADS_PER_CORE = 4   # one batch per core
HP = 2               # head pairs per core (2 heads of 64 stacked -> 128)

TA = 512             # stripe width (queries per stripe)
N_SC = TK // 128     # 16 s-chunks
N_FC = D // 128      # 8 f-chunks

DT = BF16

_CACHED = {}


def build_program():
    nc = bacc.Bacc(
        "TRN2", target_bir_lowering=False, debug=False, num_devices=N_CORES
    )

    xt_to = nc.dram_tensor("xt_to", [D, TQ], DT, kind="ExternalInput")
    xt_from = nc.dram_tensor("xt_from", [D, TK], DT, kind="ExternalInput")
    # weights already in partition-major [128, fc, d] layout host-side so
    # the DMA moves 4KB-contiguous per-partition lines (fast packets)
    wq = nc.dram_tensor("wq", [128, N_FC, 256], DT, kind="ExternalInput")
    wk = nc.dram_tensor("wk", [128, N_FC, 256], DT, kind="ExternalInput")
    wv = nc.dram_tensor("wv", [128, N_FC, 260], DT, kind="ExternalInput")
    bq = nc.dram_tensor("bq", [128, 2], F32, kind="ExternalInput")
    bk = nc.dram_tensor("bk", [128, 2], F32, kind="ExternalInput")
    bv = nc.dram_tensor("bv", [1, 260], DT, kind="ExternalInput")
    wot = nc.dram_tensor("wot", [128, 2, 1024], DT, kind="ExternalInput")
    out = nc.dram_tensor("out", [TQ, D], DT, kind="ExternalOutput")

    with tile.TileContext(nc) as tc:
        with (
            tc.tile_pool(name="wpool", bufs=1) as wpool,
            tc.tile_pool(name="actpool", bufs=1) as actpool,
            tc.tile_pool(name="ptpool", bufs=4) as ptpool,
            tc.tile_pool(name="misc", bufs=2) as misc,
            tc.tile_pool(name="psmm", bufs=2, space="PSUM") as psmm,
            tc.tile_pool(name="psacc", bufs=2, space="PSUM") as psacc,
            tc.tile_pool(name="psaux", bufs=2, space="PSUM") as psaux,
        ):
            # ---- weights / constants -------------------------------------
            wq_sb = wpool.tile([128, N_FC, 256], DT)
            wk_sb = wpool.tile([128, N_FC, 256], DT)
            wv_sb = wpool.tile([128, N_FC, 260], DT)
            bq_sb = wpool.tile([128, 2], F32)
            bk_sb = wpool.tile([128, 2], F32)
            bv_sb = wpool.tile([1, 260], DT)
            wot_sb = wpool.tile([128, 2, 1024], DT)
            ones_sb = wpool.tile([1, 128], DT)
            nc.vector.memset(ones_sb[:], 1.0)

            # x^T chunk-pair tiles -> DMA->fill-thunk deps at matching
            # granularity (each fill thunk consumes one fc pair)
            xfr_sb = [
                actpool.tile([128, 2, TK], DT, name=f"xfr{fp}")
                for fp in range(N_FC // 2)
            ]
            xto_sb = [
                actpool.tile([128, 2, TQ], DT, name=f"xto{fp}")
                for fp in range(N_FC // 2)
            ]
            xt_to_r = xt_to.rearrange("(c p) t -> p c t", p=128)
            xt_from_r = xt_from.rearrange("(c p) t -> p c t", p=128)

            # Input DMAs split across BOTH hardware DGE queues: the x_from
            # (K/V) stream issues from Sync, the x_to (Q) stream from the
            # Scalar engine (idle until the first exp anyway), so the two
            # 4MB streams transfer in parallel.  Biases trail the x data.
            nc.sync.dma_start(wk_sb[:], wk[:])
            nc.sync.dma_start(wv_sb[:], wv[:])
            nc.scalar.dma_start(wq_sb[:], wq[:])
            # all x data sequentially on the sync queue, x_from first: the
            # two streams share HBM bandwidth anyway, and K/V work unblocks
            # the PE earliest (xto's tail overlaps the K/V prologue)
            for fp in range(N_FC // 2):
                nc.sync.dma_start(
                    xfr_sb[fp][:], xt_from_r[:, 2 * fp:2 * fp + 2, :]
                )
            for fp in range(N_FC // 2):
                nc.sync.dma_start(
                    xto_sb[fp][:], xt_to_r[:, 2 * fp:2 * fp + 2, :]
                )
            nc.sync.dma_start(bk_sb[:], bk[:])
            nc.sync.dma_start(bv_sb[:], bv[:])
            nc.scalar.dma_start(bq_sb[:], bq[:])
            nc.scalar.dma_start(wot_sb[:], wot[:])

            # ---- persistent activations ----------------------------------
            qt_sb = [
                actpool.tile([128, TQ], DT, name=f"qt{hp}") for hp in range(HP)
            ]
            kt_sb = [
                actpool.tile([128, TK], DT, name=f"kt{hp}") for hp in range(HP)
            ]
            # V^T with ones columns: head h at cols 65h..65h+63, ones at
            # 65h+64 (4 heads -> 260 cols), per 128-wide s-chunk
            vn_sb = actpool.tile([128, N_SC, 260], DT, name="vn_sb")
            ot_sb = [
                actpool.tile([128, TQ], DT, name=f"ot{hp}") for hp in range(HP)
            ]

            # ---- thunk emitters ------------------------------------------
            def qk_thunks(w_sb, b_sb, x_sb, dst, hp, tt):
                """Q/K projection for one [128, 1024] tile: two psum halves,
                each 8 accumulating matmuls + a bias-add copyback."""
                thunks = []
                dsl = bass.ts(hp, 128)
                for half in range(2):
                    ps = psaux.tile([128, 512], F32, name="ps_x")
                    t0 = tt * 1024 + half * 512
                    for fcp in range(N_FC // 2):
                        def fill(fcp=fcp, ps=ps, t0=t0):
                            for k in range(2):
                                fc = 2 * fcp + k
                                nc.tensor.matmul(
                                    ps[:],
                                    w_sb[:, fc, dsl],
                                    x_sb[fcp][:, k, t0:t0 + 512],
                                    start=(fc == 0),
                                    stop=(fc == N_FC - 1),
                                )
                        thunks.append(fill)

                    def copyback(ps=ps, t0=t0):
                        nc.vector.tensor_scalar_add(
                            dst[hp][:, t0:t0 + 512], ps[:], b_sb[:, hp:hp + 1]
                        )
                    thunks.append(copyback)
                return thunks

            def v_thunks(sc):
                """V^T for one s-chunk, computed directly in [s, dh]
                orientation: stationary = x_from^T chunk, moving = Wv.
                Bias + ones columns injected via a K=1 matmul."""
                thunks = []
                ps = psaux.tile([128, 512], F32, name="ps_x")
                ssl = bass.ts(sc, 128)
                for fcp in range(N_FC // 2):
                    def fill(fcp=fcp, ps=ps):
                        for k in range(2):
                            fc = 2 * fcp + k
                            nc.tensor.matmul(
                                ps[:, 0:260],
                                xfr_sb[fcp][:, k, ssl],
                                wv_sb[:, fc, :],
                                start=(fc == 0),
                                stop=False,
                            )
                    thunks.append(fill)

                def bias(ps=ps):
                    nc.tensor.matmul(
                        ps[:, 0:260],
                        ones_sb[:],
                        bv_sb[:],
                        start=False,
                        stop=True,
                    )
                thunks.append(bias)

                def copyback(ps=ps):
                    nc.vector.tensor_copy(vn_sb[:, sc, :], ps[:, 0:260])
                thunks.append(copyback)
                return thunks

            out_r = out.rearrange("(a p) d -> p a d", p=128)

            def outproj_thunks(tta):
                """Output projection for one stripe of queries: 4 t-chunks
                of 128, each = 2 psum halves (contraction over both head
                pairs) + copyback into a stripe-wide staging tile, then one
                batched DMA for all 512 rows."""
                thunks = []
                o_t = misc.tile([128, TA // 128, 1024], DT, name="o_t")
                for j in range(TA // 128):
                    tc_ = tta * (TA // 128) + j
                    tsl = bass.ts(tc_, 128)
                    for half in range(2):
                        ps = psaux.tile([128, 512], F32, name="ps_x")
                        hsl = bass.ts(half, 512)

                        def mmf(ps=ps, tsl=tsl, hsl=hsl):
                            for hp in range(HP):
                                nc.tensor.matmul(
                                    ps[:],
                                    ot_sb[hp][:, tsl],
                                    wot_sb[:, hp, hsl],
                                    start=(hp == 0),
                                    stop=(hp == HP - 1),
                                )
                        thunks.append(mmf)

                        def cb(ps=ps, j=j, hsl=hsl):
                            nc.vector.tensor_copy(o_t[:, j, hsl], ps[:])
                        thunks.append(cb)

                def store():
                    nc.sync.dma_start(
                        out_r[:, 4 * tta:4 * tta + 4, :], o_t[:]
                    )
                thunks.append(store)
                return thunks

            def emit_stripe(tta, hp, pop_filler, per_iter):
                """One attention stripe: both heads of the pair, 512
                queries, all 2048 keys.  Pops filler thunks per s-chunk so
                independent PE work interleaves with the ACT exp stream.
                ps_o is split per head on a bufs=2 ring so the next
                stripe's PV only waits on the matching head's drain."""
                ps_o = [
                    psacc.tile([65, TA], F32, name="ps_o") for _ in range(2)
                ]
                for sc in range(N_SC):
                    ps_s = psmm.tile([128, 1024], F32, name="ps_s")
                    for h in range(2):
                        hb = 64 * h
                        nc.tensor.matmul(
                            ps_s[:, bass.ts(h, TA)],
                            kt_sb[hp][hb:hb + 64, bass.ts(sc, 128)],
                            qt_sb[hp][hb:hb + 64, bass.ts(tta, TA)],
                            start=True,
                            stop=True,
                        )
                    pt = ptpool.tile([128, 1024], DT, name="pt")
                    nc.scalar.activation(pt[:], ps_s[:], AF.Exp)
                    # one filler BETWEEN scores and PV: the PE has
                    # guaranteed-ready work while this iteration's exp runs
                    pop_filler(1)
                    for h in range(2):
                        vb = 65 * (2 * hp + h)
                        nc.tensor.matmul(
                            ps_o[h][:],
                            vn_sb[:, sc, vb:vb + 65],
                            pt[:, bass.ts(h, TA)],
                            start=(sc == 0),
                            stop=(sc == N_SC - 1),
                        )
                    pop_filler(per_iter - 1)

                # denominators (psum row 64) -> fast reciprocal -> broadcast
                # across partitions (Pool) -> normalize ps_o into ot (DVE)
                # (reciprocal_approx_fast must NOT read PSUM directly)
                recs = []
                for h in range(2):
                    rec = misc.tile([1, TA], F32, name="rec_t")
                    nc.vector.tensor_copy(rec[:], ps_o[h][64:65, :])
                    nc.vector.reciprocal_approx_fast(rec[:], rec[:])
                    recs.append(rec)
                for h in range(2):
                    r_sb = misc.tile([128, TA], F32, name="r_sb")
                    nc.gpsimd.partition_broadcast(r_sb[:], recs[h][:])
                    hb = 64 * h
                    nc.vector.tensor_mul(
                        ot_sb[hp][hb:hb + 64, bass.ts(tta, TA)],
                        ps_o[h][0:64, :],
                        r_sb[0:64, :],
                    )

            # ---- emission schedule ---------------------------------------
            # Minimal prologue so the exp stream starts as early as the
            # input DMA allows: K(hp0,tt0) covers keys for s-chunks 0..7,
            # V(0..7), Q(hp0,tt0).  Everything else (K tt1, V 8..15, the
            # other head pair, remaining Q tiles, output projections)
            # streams in as filler thunks inside the stripes.  A stripe's
            # own s-chunk loop only consumes V(sc)/K(tt1) at iteration
            # sc >= 8, by which point the first stripe's high filler rate
            # has emitted them.  Anything a stripe needs at its FIRST
            # iteration is force-drained before the stripe starts.
            for f in qk_thunks(wk_sb, bk_sb, xfr_sb, kt_sb, 0, 0):
                f()
            for sc in range(8):
                for f in v_thunks(sc):
                    f()
            for f in qk_thunks(wq_sb, bq_sb, xto_sb, qt_sb, 0, 0):
                f()

            fillers = deque()
            fillers.extend(qk_thunks(wk_sb, bk_sb, xfr_sb, kt_sb, 0, 1))
            for sc in range(8, N_SC):
                fillers.extend(v_thunks(sc))
            fillers.extend(qk_thunks(wk_sb, bk_sb, xfr_sb, kt_sb, 1, 0))
            fillers.extend(qk_thunks(wk_sb, bk_sb, xfr_sb, kt_sb, 1, 1))
            fillers.extend(qk_thunks(wq_sb, bq_sb, xto_sb, qt_sb, 1, 0))
            n_before_01 = len(fillers)
            fillers.extend(qk_thunks(wq_sb, bq_sb, xto_sb, qt_sb, 0, 1))
            n_before_10 = len(fillers)
            fillers.extend(qk_thunks(wq_sb, bq_sb, xto_sb, qt_sb, 1, 1))
            n_before_11 = len(fillers)

            popped = [0]

            def pop_filler(n):
                for _ in range(n):
                    if fillers:
                        fillers.popleft()()
                        popped[0] += 1

            def drain_to(target):
                while fillers and popped[0] < target:
                    fillers.popleft()()
                    popped[0] += 1

            per_iter_schedule = {
                (0, 0): 7, (0, 1): 3, (1, 0): 3, (1, 1): 2,
                (2, 0): 2, (2, 1): 2, (3, 0): 2, (3, 1): 2,
            }
            guards = {(0, 1): n_before_01, (1, 0): n_before_10,
                      (1, 1): n_before_11}

            for tta in range(TQ // TA):
                for hp in range(HP):
                    drain_to(guards.get((tta, hp), 0))
                    emit_stripe(tta, hp, pop_filler,
                                per_iter_schedule[(tta, hp)])
                fillers.extend(outproj_thunks(tta))

            while fillers:
                fillers.popleft()()

    nc.compile()
    return nc


def _prep_in_maps(x_to, x_from, Wq, bq, Wk, bk, Wv, bv, Wo):
    scale = 1.0 / np.sqrt(np.float32(DH))
    # [H, D, DH] -> [D, H*DH] with column h*DH+d
    wq_f = np.ascontiguousarray(Wq.transpose(1, 0, 2).reshape(D, H * DH)) * scale
    wk_f = np.ascontiguousarray(Wk.transpose(1, 0, 2).reshape(D, H * DH))
    bq_f = bq.reshape(H * DH) * scale
    bk_f = bk.reshape(H * DH)

    xt_to = np.ascontiguousarray(x_to.transpose(0, 2, 1))    # [B, D, TQ]
    xt_from = np.ascontiguousarray(x_from.transpose(0, 2, 1))

    def f32(a):
        return np.ascontiguousarray(a, dtype=np.float32)

    import ml_dtypes

    def fdt(a):
        return np.ascontiguousarray(a, dtype=ml_dtypes.bfloat16)

    in_maps = []
    for c in range(N_CORES):
        b, g = divmod(c, HEADS_PER_CORE)
        cs = slice(g * 256, (g + 1) * 256)
        # Wv augmented: head h (of the core's 4) at cols 65h..65h+63,
        # zero col at 65h+64; bias row gets bv there plus 1.0 ones
        wv_aug = np.zeros((D, 260), dtype=np.float32)
        bv_aug = np.zeros((260,), dtype=np.float32)
        for h in range(4):
            head = 4 * g + h
            wv_aug[:, 65 * h:65 * h + 64] = Wv[head]
            bv_aug[65 * h:65 * h + 64] = bv[head]
            bv_aug[65 * h + 64] = 1.0
        def pmajor(w):
            # [D, d] -> [128, N_FC, d] partition-major
            return np.ascontiguousarray(
                w.reshape(N_FC, 128, w.shape[1]).transpose(1, 0, 2)
            )

        in_maps.append(
            {
                "xt_to": fdt(xt_to[b]),
                "xt_from": fdt(xt_from[b]),
                "wq": fdt(pmajor(wq_f[:, cs])),
                "wk": fdt(pmajor(wk_f[:, cs])),
                "wv": fdt(pmajor(wv_aug)),
                # [256] -> [2 pairs, 128] -> [128, 2]
                "bq": f32(bq_f[cs].reshape(2, 128).T),
                "bk": f32(bk_f[cs].reshape(2, 128).T),
                "bv": fdt(bv_aug.reshape(1, 260)),
                # Wo[:, cs].T = [256, 1024] -> [2, 128, 1024] -> [128, 2, 1024]
                "wot": fdt(
                    np.ascontiguousarray(Wo[:, cs].T)
                    .reshape(2, 128, 1024)
                    .transpose(1, 0, 2)
                ),
            }
        )
    return in_maps


LAST_EXEC_TIME_NS = None
LAST_TRACE = None


def kernel(x_to, x_from, Wq, bq, Wk, bk, Wv, bv, Wo, bo):
    global LAST_EXEC_TIME_NS, LAST_TRACE
    if "nc" not in _CACHED:
        _CACHED["nc"] = build_program()
    nc = _CACHED["nc"]

    in_maps = _prep_in_maps(
        np.asarray(x_to), np.asarray(x_from), np.asarray(Wq), np.asarray(bq),
        np.asarray(Wk), np.asarray(bk), np.asarray(Wv), np.asarray(bv),
        np.asarray(Wo),
    )
    res = run_bass_kernel_spmd(nc, in_maps, list(range(N_CORES)))
    LAST_EXEC_TIME_NS = res.exec_time_ns
    LAST_TRACE = res.instructions_and_trace

    out = np.zeros((B, TQ, D), dtype=np.float32)
    for c in range(N_CORES):
        out[c // HEADS_PER_CORE] += np.asarray(
            res.results[c]["out"], dtype=np.float32
        )
    out += np.asarray(bo, dtype=np.float32)
    return out
